# revision 12
# baseline (speedup 1.0000x reference)
"""Causal attention head + residual + LayerNorm on 8 TRN2 NeuronCores.

Problem (S=8192, D=512, f32):
    q = x @ Wq.T ; k = x @ Wk.T ; v = x @ Wv.T
    scores = (q @ k.T) / sqrt(D) + causal_mask * (-1e9)
    attn = softmax(scores, axis=1)
    out = LayerNorm(attn @ v + q) * gamma + beta

Strategy (no collectives):
  - Interleaved row sharding: core m owns query rows m::8.  With this
    sharding the causal block structure is identical on every core
    (tile u of 128 local rows needs exactly key-blocks 0..8u+7), so one
    static SPMD graph fits all cores; the fine-grained (stride-8) causal
    boundary is handled by per-core 0/1 mask tiles supplied as input data.
  - Scores are computed TRANSPOSED: sT[sk, sq] = kT.T @ qT, so the
    exp() output pT[sk, sq] is directly the lhsT operand of the AV
    matmul -- no on-chip transposes anywhere.
  - Softmax skips the max-subtraction (scores are O(10); exp stays well
    inside f32/bf16 range) and gets row sums from an extra ones-column
    matmul; normalization (1/sum), +q residual and LayerNorm run as a
    fused epilogue.
  - Each core computes full kT/v redundantly (collectives measure only
    ~62 GB/s -- an AllGather of K/V would cost far more than the
    replicated 4.3 GMAC of PE time).
  - Matmul inputs in bf16 (f32 PSUM accumulation); mask is never
    materialized/loaded (the 256 MB mask input is ignored; causality is
    reproduced exactly, including the -1e9 saturation to 0.0 in f32).
"""

import math

import numpy as np
import ml_dtypes

S = 8192
D = 512
P = 128
DT = D // P          # 4 d-slices
NCORES = 8
LOCAL = S // NCORES  # 1024 local rows
NT = LOCAL // P      # 8 local sq tiles
NB = S // P          # 64 sk blocks
EPS = 1e-5
SCALE = 1.0 / math.sqrt(D)

BF16 = ml_dtypes.bfloat16

_CACHE = {}

# exec time of the last hardware run (ns), for test harnesses
LAST_EXEC_NS = None


def _build_graph():
    import concourse.bass as bass
    import concourse.tile as tile
    from concourse import mybir

    nc = bass.Bass("TRN2")
    f32 = mybir.dt.float32
    bf16 = mybir.dt.bfloat16

    xT_p = nc.declare_dram_parameter("xT", [D, S], bf16, isOutput=False)
    xTl_p = nc.declare_dram_parameter("xTl", [D, LOCAL], bf16, isOutput=False)
    wq_p = nc.declare_dram_parameter("wqT", [D, D], bf16, isOutput=False)
    wk_p = nc.declare_dram_parameter("wkT", [D, D], bf16, isOutput=False)
    wv_p = nc.declare_dram_parameter("wvT", [D, D], bf16, isOutput=False)
    msk_p = nc.declare_dram_parameter("masks", [P, 2, 512], bf16, isOutput=False)
    gam_p = nc.declare_dram_parameter("gamma", [D], f32, isOutput=False)
    bet_p = nc.declare_dram_parameter("beta", [D], f32, isOutput=False)
    out_p = nc.declare_dram_parameter("out", [LOCAL, D], f32, isOutput=True)

    # DRAM views with the partition dim innermost on rows: [p, dt, cols]
    xT_r = xT_p[:].rearrange("(dt p) s -> p dt s", p=P)
    xTl_r = xTl_p[:].rearrange("(dt p) s -> p dt s", p=P)
    wq_r = wq_p[:].rearrange("(dt p) s -> p dt s", p=P)
    wk_r = wk_p[:].rearrange("(dt p) s -> p dt s", p=P)
    wv_r = wv_p[:].rearrange("(dt p) s -> p dt s", p=P)

    def bcast(ap, parts=P):
        return bass.AP(
            tensor=ap.tensor, offset=ap.offset, ap=[[0, parts]] + list(ap.ap)
        )

    Exp = mybir.ActivationFunctionType.Exp
    Sqrt = mybir.ActivationFunctionType.Sqrt
    sub = mybir.AluOpType.subtract
    mult = mybir.AluOpType.mult

    with tile.TileContext(nc) as tc:
        from contextlib import ExitStack

        with ExitStack() as ctx:
            const = ctx.enter_context(tc.tile_pool(name="const", bufs=1))
            xt_pool = ctx.enter_context(tc.tile_pool(name="xt", bufs=2))
            pt_pool = ctx.enter_context(tc.tile_pool(name="pt", bufs=3))
            ep_pool = ctx.enter_context(tc.tile_pool(name="ep", bufs=3))
            small = ctx.enter_context(tc.tile_pool(name="small", bufs=6))
            mm_ps = ctx.enter_context(tc.tile_pool(name="mm", bufs=2, space="PSUM"))
            st_ps = ctx.enter_context(tc.tile_pool(name="st", bufs=2, space="PSUM"))
            av_ps = ctx.enter_context(tc.tile_pool(name="av", bufs=2, space="PSUM"))
            sm_ps = ctx.enter_context(tc.tile_pool(name="sm", bufs=2, space="PSUM"))

            # ---- constants / persistent tensors ----
            wq = const.tile([P, DT, D], bf16)
            wk = const.tile([P, DT, D], bf16)
            wv = const.tile([P, DT, D], bf16)
            xTl = const.tile([P, DT, LOCAL], bf16)
            masks = const.tile([P, 2, 512], bf16)
            gam_bc = const.tile([P, D], f32)
            bet_bc = const.tile([P, D], f32)
            eps_t = const.tile([P, 1], f32)
            zero_t = const.tile([P, 1], f32)
            ones_t = const.tile([P, 1], bf16)
            kT = const.tile([P, DT, S], bf16)       # 64 KB/partition
            vsb = const.tile([P, NB, D], bf16)      # 64 KB/partition
            qT = const.tile([P, DT, LOCAL], bf16)
            qres = const.tile([P, NT, D], f32)      # q rows for the residual
            wacc = const.tile([P, NT, D], bf16)     # pre-LN activations
            mv = const.tile([P, NT, 2], f32)        # per-tile mean/var

            nc.sync.dma_start(out=wq, in_=wq_r)
            nc.sync.dma_start(out=wk, in_=wk_r)
            nc.sync.dma_start(out=wv, in_=wv_r)
            nc.sync.dma_start(out=xTl, in_=xTl_r)
            nc.sync.dma_start(out=masks, in_=msk_p[:])
            nc.sync.dma_start(out=gam_bc, in_=bcast(gam_p[:]))
            nc.sync.dma_start(out=bet_bc, in_=bcast(bet_p[:]))
            nc.vector.memset(eps_t, EPS)
            nc.vector.memset(zero_t, 0.0)
            nc.vector.memset(ones_t, 1.0)

            # Make PE observe the wk/wv DMA semaphores early; otherwise the
            # first kv matmul's fused LDWEIGHTS+MM carries 3 sync waits,
            # which overflows the 2-slot MM wait limit in walrus codegen.
            nc.tensor.ldweights(wk[:, 0, 0:P])
            nc.tensor.ldweights(wv[:, 0, 0:P])

            # ---- q (residual, normal layout) and qT ----
            for t in range(NT):
                ps = mm_ps.tile([P, D], f32)
                for di in range(DT):
                    nc.tensor.matmul(
                        ps,
                        lhsT=xTl[:, di, t * P:(t + 1) * P],
                        rhs=wq[:, di, :],
                        start=(di == 0),
                        stop=(di == DT - 1),
                    )
                nc.scalar.copy(qres[:, t, :], ps)
            for dt in range(DT):
                for h in range(2):
                    ps = mm_ps.tile([P, D], f32)
                    for di in range(DT):
                        nc.tensor.matmul(
                            ps,
                            lhsT=wq[:, di, dt * P:(dt + 1) * P],
                            rhs=xTl[:, di, h * 512:(h + 1) * 512],
                            start=(di == 0),
                            stop=(di == DT - 1),
                        )
                    nc.vector.tensor_copy(qT[:, dt, h * 512:(h + 1) * 512], ps)

            # ---- main loop: kv column blocks interleaved with attention ----
            for u in range(NT):
                for c in (2 * u, 2 * u + 1):
                    xt = xt_pool.tile([P, DT, 512], bf16)
                    # WAW touch absorbs the PE reader wait so the DMA itself
                    # carries only its DMA-lane wait (1-wait HW limit).
                    nc.gpsimd.memset(xt[0:1, 0:1, 0:1], 0.0)
                    nc.gpsimd.dma_start(out=xt, in_=xT_r[:, :, c * 512:(c + 1) * 512])
                    # absorb the DMA wait into a standalone LDWEIGHTS so the
                    # matmuls below carry at most one sync wait (HW MM limit)
                    nc.tensor.ldweights(xt[:, 0, 0:P])
                    for dt in range(DT):  # kT block
                        ps = mm_ps.tile([P, D], f32)
                        for di in range(DT):
                            nc.tensor.matmul(
                                ps,
                                lhsT=wk[:, di, dt * P:(dt + 1) * P],
                                rhs=xt[:, di, :],
                                start=(di == 0),
                                stop=(di == DT - 1),
                            )
                        nc.vector.tensor_copy(kT[:, dt, c * 512:(c + 1) * 512], ps)
                    for sv in range(4):  # v rows 128*(4c+sv)
                        ps = mm_ps.tile([P, D], f32)
                        for di in range(DT):
                            nc.tensor.matmul(
                                ps,
                                lhsT=xt[:, di, sv * P:(sv + 1) * P],
                                rhs=wv[:, di, :],
                                start=(di == 0),
                                stop=(di == DT - 1),
                            )
                        nc.scalar.copy(vsb[:, 4 * c + sv, :], ps)

                # attention for sq tile u  (key blocks 0 .. 8u+7)
                av = av_ps.tile([P, D], f32)
                sm = sm_ps.tile([P, 1], f32)
                nj = 8 * u + 8
                for g in range(2 * u + 2):
                    st = st_ps.tile([P, 512], f32)
                    for jj in range(4):
                        j = 4 * g + jj
                        for dt in range(DT):
                            nc.tensor.matmul(
                                st[:, jj * P:(jj + 1) * P],
                                lhsT=kT[:, dt, j * P:(j + 1) * P],
                                rhs=qT[:, dt, u * P:(u + 1) * P],
                                start=(dt == 0),
                                stop=(dt == DT - 1),
                            )
                    pt = pt_pool.tile([P, 512], bf16)
                    nc.scalar.activation(pt, st, Exp, bias=zero_t, scale=SCALE)
                    if g >= 2 * u:  # diagonal band: apply causal mask
                        nc.vector.tensor_mul(pt, pt, masks[:, g - 2 * u, :])
                    for jj in range(4):
                        j = 4 * g + jj
                        nc.tensor.matmul(
                            av,
                            lhsT=pt[:, jj * P:(jj + 1) * P],
                            rhs=vsb[:, j, :],
                            start=(j == 0),
                            stop=(j == nj - 1),
                        )
                        nc.tensor.matmul(
                            sm,
                            lhsT=pt[:, jj * P:(jj + 1) * P],
                            rhs=ones_t,
                            start=(j == 0),
                            stop=(j == nj - 1),
                        )

                # epilogue: w = av/sum + q ; save stats, defer LN scale
                r = small.tile([P, 1], f32)
                nc.vector.reciprocal(r, sm)
                w = ep_pool.tile([P, D], f32, tag="w")
                nc.vector.tensor_scalar_mul(w, av, r)
                nc.vector.tensor_add(w, w, qres[:, u, :])
                stats = small.tile([P, 6], f32)
                nc.vector.bn_stats(stats, w)
                nc.vector.bn_aggr(mv[:, u, :], stats)
                nc.vector.tensor_copy(wacc[:, u, :], w)

            # ---- final pass: LayerNorm application (one Sqrt table load) ----
            for u in range(NT):
                std = small.tile([P, 1], f32)
                nc.scalar.activation(std, mv[:, u, 1:2], Sqrt, bias=eps_t, scale=1.0)
                rstd = small.tile([P, 1], f32)
                nc.vector.reciprocal(rstd, std)
                w2 = ep_pool.tile([P, D], f32, tag="w")
                nc.vector.tensor_scalar(
                    w2, wacc[:, u, :], mv[:, u, 0:1], rstd, op0=sub, op1=mult
                )
                nc.vector.tensor_mul(w2, w2, gam_bc)
                nc.vector.tensor_add(w2, w2, bet_bc)
                # read-touch absorbs the DVE wait on the issuing engine
                scr = small.tile([P, 1], f32, tag="scr")
                nc.gpsimd.tensor_copy(scr[0:1, 0:1], w2[0:1, 0:1])
                nc.gpsimd.dma_start(out=out_p[u * P:(u + 1) * P, :], in_=w2)

    _limit_waits(nc)
    return nc


def _limit_waits(nc, max_waits: int = 1):
    """This toolchain's walrus accepts at most one sync wait per engine
    instruction. Split any instruction with more: hoist the extra waits
    onto same-engine InstNoOp carriers inserted immediately before it
    (the engine stalls at the NoOp instead -- semantically identical)."""
    from concourse import mybir

    for f in nc.m.functions:
        for bb in f.blocks:
            il = bb.instructions
            idx = 0
            while idx < len(il):
                i = il[idx]
                si = i.sync_info
                waits = list(si.on_wait) if (si and si.on_wait) else []
                if len(waits) > max_waits:
                    extra = waits[:-max_waits]
                    keep = waits[-max_waits:]
                    for w in extra:
                        nop = mybir.InstNoOp(
                            name=f"{i.name}-w{w.ant_name}", ins=[], outs=[]
                        )
                        nop.engine = i.engine
                        nop.sync_info = type(si)(on_wait=[w], on_update=[])
                        il.insert(idx, nop)
                        idx += 1
                    i.sync_info = type(si)(on_wait=keep, on_update=si.on_update)
                idx += 1


def _host_masks(m: int) -> np.ndarray:
    """[128, 2, 512] bf16; [p, gg, 128*q + f] = 1.0 iff key-col offset p of
    diagonal block jj = 4*gg+q is visible to query row m + 8*(128*u + f)."""
    p = np.arange(P)[:, None, None]
    jj = np.arange(8)[None, :, None]
    f = np.arange(P)[None, None, :]
    msk = (p <= 8 * f + (m - 128 * jj)).astype(np.float32)
    return msk.reshape(P, 2, 512).astype(BF16)


def kernel(x, mask, Wq, Wk, Wv, gamma, beta):
    global LAST_EXEC_NS
    from concourse.bass_utils import run_bass_kernel_spmd

    if "nc" not in _CACHE:
        _CACHE["nc"] = _build_graph()
    nc = _CACHE["nc"]

    x = np.asarray(x, dtype=np.float32)
    xT = np.ascontiguousarray(x.T).astype(BF16)
    wqT = np.ascontiguousarray(np.asarray(Wq, np.float32).T).astype(BF16)
    wkT = np.ascontiguousarray(np.asarray(Wk, np.float32).T).astype(BF16)
    wvT = np.ascontiguousarray(np.asarray(Wv, np.float32).T).astype(BF16)
    gam = np.ascontiguousarray(np.asarray(gamma, np.float32))
    bet = np.ascontiguousarray(np.asarray(beta, np.float32))

    in_maps = []
    for m in range(NCORES):
        xTl = np.ascontiguousarray(x[m::NCORES].T).astype(BF16)
        in_maps.append(
            {
                "xT": xT,
                "xTl": xTl,
                "wqT": wqT,
                "wkT": wkT,
                "wvT": wvT,
                "masks": _host_masks(m),
                "gamma": gam,
                "beta": bet,
            }
        )

    trace = bool(int(__import__("os").environ.get("KERNEL_TRACE", "0")))
    res = run_bass_kernel_spmd(
        nc, in_maps, core_ids=list(range(NCORES)), trace=trace
    )
    LAST_EXEC_NS = getattr(res, "exec_time_ns", None)

    out = np.empty((S, D), dtype=np.float32)
    for m in range(NCORES):
        out[m::NCORES] = res.results[m]["out"]
    return out
